# revision 16
# baseline (speedup 1.0000x reference)
"""Trainium2 Bass kernel for CTM (cluster-dpc-knn token merge), 8-core SPMD.

Pipeline (per batch b, rows split across 2 cores):
  NEFF A: e_ij = sq_j - 2*x_i.x_j  (augmented matmul), 8-NN d2 sum (max8) and
          rowwise max of e  -> density ingredients.
  host:   density = exp(-sum9/(9*256)) + threefry-noise*1e-6 ; dist_max.
  NEFF B: masked min over e where density_j > density_i  -> parent distance.
  host:   score = parent_dist*density ; top-128 centers (stable, desc).
  NEFF C: per-token argmin over 128 centers + one-hot segment-sum merge matmul.
  host:   combine halves, divide by (weight sum + 1e-6).
"""
import numpy as np

import concourse.bass as bass
import concourse.tile as tile
from concourse import bacc, mybir
from concourse.bass_utils import run_bass_kernel_spmd

B, N, C, M = 4, 5120, 256, 128
HALF = N // 2            # rows per core
NB = HALF // 128         # 20 row blocks per core
JT = N // 512            # 10 column tiles
BIGF = 1.0e9
FP = mybir.dt.float32
AX = mybir.AxisListType
OP = mybir.AluOpType
AF = mybir.ActivationFunctionType

MM_DTYPE = mybir.dt.float32


def _mm(nc, out, lhsT, rhs, **kw):
    nc.tensor.matmul(out, lhsT.bitcast(MM_DTYPE), rhs.bitcast(MM_DTYPE), **kw)


# ----------------------------------------------------------------- threefry
def _rotl(x, r):
    return (x << np.uint32(r)) | (x >> np.uint32(32 - r))


def _noise(shape):
    """Bit-exact replica of jax.random.uniform(jax.random.key(1), shape, f32)."""
    n = int(np.prod(shape))
    counts = np.arange(n, dtype=np.uint32)
    x0, x1 = counts[: n // 2].copy(), counts[n // 2 :].copy()
    k0, k1 = np.uint32(0), np.uint32(1)
    ks = [k0, k1, np.uint32(0x1BD11BDA) ^ k0 ^ k1]
    rot = [[13, 15, 26, 6], [17, 29, 16, 24]]
    with np.errstate(over="ignore"):
        x0 = x0 + ks[0]
        x1 = x1 + ks[1]
        for i in range(5):
            for r in rot[i % 2]:
                x0 = x0 + x1
                x1 = _rotl(x1, r)
                x1 = x1 ^ x0
            x0 = x0 + ks[(i + 1) % 3]
            x1 = x1 + ks[(i + 2) % 3] + np.uint32(i + 1)
    bits = np.concatenate([x0, x1])
    fbits = (bits >> np.uint32(9)) | np.uint32(0x3F800000)
    return (fbits.view(np.float32) - np.float32(1.0)).reshape(shape)


# ----------------------------------------------------------------- NEFF A
def build_a():
    nc = bacc.Bacc("TRN2", target_bir_lowering=False, debug=False, num_devices=8)
    xt_d = nc.dram_tensor("xt", [256, N], FP, kind="ExternalInput")
    sqj_d = nc.dram_tensor("sqj", [1, N], FP, kind="ExternalInput")
    lhs_d = nc.dram_tensor("lhs", [256, HALF], FP, kind="ExternalInput")
    sqi_d = nc.dram_tensor("sqi", [128, NB], FP, kind="ExternalInput")
    sum9_d = nc.dram_tensor("sum9", [128, NB], FP, kind="ExternalOutput")
    emin_d = nc.dram_tensor("eminneg", [128, NB], FP, kind="ExternalOutput")
    ebuf_d = nc.dram_tensor("ebuf", [NB, 128, N], FP, kind="ExternalOutput")
    with tile.TileContext(nc) as tc:
        with (
            tc.tile_pool(name="big", bufs=1) as big,
            tc.tile_pool(name="row", bufs=2) as rowp,
            tc.tile_pool(name="small", bufs=4) as sm,
            tc.tile_pool(name="psum", bufs=4, space="PSUM") as pp,
        ):
            xt0 = big.tile([128, N], FP)
            xt1 = big.tile([128, N], FP)
            sqj = big.tile([1, N], FP)
            lh0 = big.tile([128, HALF], FP)
            lh1 = big.tile([128, HALF], FP)
            sqb = big.tile([128, N], FP)
            ones1 = big.tile([1, 128], FP)
            sqi = big.tile([128, NB], FP)
            ident = big.tile([128, 128], mybir.dt.int32)
            negbig = big.tile([128, 128], FP)
            sum9sb = big.tile([128, NB], FP)
            eminsb = big.tile([128, NB], FP)
            nc.gpsimd.dma_start(xt0[:], xt_d[0:128, :])
            nc.gpsimd.dma_start(xt1[:], xt_d[128:256, :])
            nc.gpsimd.dma_start(sqj[:], sqj_d[:, :])
            nc.gpsimd.dma_start(lh0[:], lhs_d[0:128, :])
            nc.gpsimd.dma_start(lh1[:], lhs_d[128:256, :])
            nc.gpsimd.dma_start(sqi[:], sqi_d[:, :])
            iot_c = sm.tile([128, 128], FP)
            iot_p = sm.tile([128, 128], FP)
            nc.gpsimd.iota(iot_c[:], [[1, 128]], channel_multiplier=0,
                           allow_small_or_imprecise_dtypes=True)
            nc.gpsimd.iota(iot_p[:], [[0, 128]], channel_multiplier=1,
                           allow_small_or_imprecise_dtypes=True)
            nc.vector.tensor_tensor(ident[:], iot_c[:], iot_p[:], OP.is_equal)
            nc.vector.memset(negbig[:], -BIGF)
            nc.vector.memset(ones1[:], 1.0)
            # broadcast sq_j across partitions via rank-1 matmul
            for j in range(JT):
                jsl = slice(j * 512, (j + 1) * 512)
                pb = pp.tile([128, 512], FP)
                _mm(nc, pb[:], ones1[:], sqj[:, jsl], start=True, stop=True)
                nc.scalar.copy(sqb[:, jsl], pb[:])

            for k in range(NB):
                ksl = slice(k * 128, (k + 1) * 128)
                negE = rowp.tile([128, N], FP)
                for j in range(JT):
                    jsl = slice(j * 512, (j + 1) * 512)
                    ps = pp.tile([128, 512], FP)
                    _mm(nc, ps[:], lh0[:, ksl], xt0[:, jsl], start=True, stop=False)
                    _mm(nc, ps[:], lh1[:, ksl], xt1[:, jsl], start=False, stop=True)
                    # -e = 2*dot - sq_j
                    nc.vector.tensor_tensor(negE[:, jsl], ps[:], sqb[:, jsl],
                                            OP.subtract)
                emin = sm.tile([128, 1], FP)
                nc.vector.tensor_reduce(emin[:], negE[:], AX.X, OP.min)
                nc.vector.tensor_copy(eminsb[:, k : k + 1], emin[:])
                # poison self-distance (own rows sit at cols [0, HALF))
                nc.vector.copy_predicated(negE[:, ksl], ident[:], negbig[:])
                nc.gpsimd.dma_start(ebuf_d[k][:, :], negE[:])
                m8 = sm.tile([128, 8], FP)
                nc.vector.max(m8[:], negE[:])
                d28 = sm.tile([128, 8], FP)
                nc.scalar.activation(d28[:], m8[:], AF.Relu,
                                     bias=sqi[:, k : k + 1], scale=-1.0)
                nc.vector.tensor_reduce(sum9sb[:, k : k + 1], d28[:], AX.X, OP.add)
            nc.gpsimd.dma_start(sum9_d[:, :], sum9sb[:])
            nc.gpsimd.dma_start(emin_d[:, :], eminsb[:])
    nc.compile()
    return nc


# ----------------------------------------------------------------- NEFF B
def build_b():
    nc = bacc.Bacc("TRN2", target_bir_lowering=False, debug=False, num_devices=8)
    ebuf_d = nc.dram_tensor("ebuf", [NB, 128, N], FP, kind="ExternalInput")
    dens_d = nc.dram_tensor("dens", [1, N], FP, kind="ExternalInput")
    densi_d = nc.dram_tensor("densi", [128, NB], FP, kind="ExternalInput")
    pmax_d = nc.dram_tensor("pmax", [128, NB], FP, kind="ExternalOutput")
    with tile.TileContext(nc) as tc:
        with (
            tc.tile_pool(name="big", bufs=1) as big,
            tc.tile_pool(name="row", bufs=2) as rowp,
            tc.tile_pool(name="small", bufs=4) as sm,
            tc.tile_pool(name="psum", bufs=2, space="PSUM") as pp,
        ):
            densi = big.tile([128, NB], FP)
            dbc = big.tile([128, N], FP)
            ones1 = big.tile([1, 128], FP)
            densr = big.tile([1, N], FP)
            pmaxsb = big.tile([128, NB], FP)
            nc.gpsimd.dma_start(densi[:], densi_d[:, :])
            nc.gpsimd.dma_start(densr[:], dens_d[:, :])
            nc.vector.memset(ones1[:], 1.0)
            for j in range(JT):
                jsl = slice(j * 512, (j + 1) * 512)
                pb = pp.tile([128, 512], FP)
                _mm(nc, pb[:], ones1[:], densr[:, jsl], start=True, stop=True)
                nc.scalar.copy(dbc[:, jsl], pb[:])

            for k in range(NB):
                negE = rowp.tile([128, N], FP)
                nc.gpsimd.dma_start(negE[:], ebuf_d[k][:, :])
                tmask = rowp.tile([128, N], FP)
                # (density_j <= density_i) * BIG
                nc.vector.tensor_scalar(tmask[:], dbc[:], densi[:, k : k + 1],
                                        BIGF, OP.is_le, OP.mult)
                u = rowp.tile([128, N], FP)
                nc.vector.tensor_tensor(u[:], negE[:], tmask[:], OP.subtract)
                # parent_e = -max(u)
                nc.vector.tensor_reduce(pmaxsb[:, k : k + 1], u[:], AX.X, OP.max)
            nc.gpsimd.dma_start(pmax_d[:, :], pmaxsb[:])
    nc.compile()
    return nc


# ----------------------------------------------------------------- NEFF C
def build_c():
    nc = bacc.Bacc("TRN2", target_bir_lowering=False, debug=False, num_devices=8)
    xth_d = nc.dram_tensor("xth", [256, HALF], FP, kind="ExternalInput")
    cxa_d = nc.dram_tensor("cxa", [257, 128], FP, kind="ExternalInput")
    xr_d = nc.dram_tensor("xr", [HALF, 256], FP, kind="ExternalInput")
    wcol_d = nc.dram_tensor("wcol", [128, NB], FP, kind="ExternalInput")
    msum_d = nc.dram_tensor("msum", [128, 257], FP, kind="ExternalOutput")
    with tile.TileContext(nc) as tc:
        with (
            tc.tile_pool(name="big", bufs=1) as big,
            tc.tile_pool(name="blk", bufs=3) as blk,
            tc.tile_pool(name="small", bufs=6) as sm,
            tc.tile_pool(name="psum", bufs=4, space="PSUM") as pp,
            tc.tile_pool(name="psacc", bufs=1, space="PSUM") as pacc,
        ):
            xth0 = big.tile([128, HALF], FP)
            xth1 = big.tile([128, HALF], FP)
            cx0 = big.tile([128, 128], FP)
            cx1 = big.tile([128, 128], FP)
            cx2 = big.tile([1, 128], FP)
            onesh = big.tile([1, HALF], FP)
            iot_c = big.tile([128, 128], FP)
            wcol = big.tile([128, NB], FP)
            msumsb = big.tile([128, 257], FP)
            nc.gpsimd.dma_start(xth0[:], xth_d[0:128, :])
            nc.gpsimd.dma_start(xth1[:], xth_d[128:256, :])
            nc.gpsimd.dma_start(cx0[:], cxa_d[0:128, :])
            nc.gpsimd.dma_start(cx1[:], cxa_d[128:256, :])
            nc.gpsimd.dma_start(cx2[:], cxa_d[256:257, :])
            nc.gpsimd.dma_start(wcol[:], wcol_d[:, :])
            nc.vector.memset(onesh[:], 1.0)
            nc.gpsimd.iota(iot_c[:], [[1, 128]], channel_multiplier=0,
                           allow_small_or_imprecise_dtypes=True)
            msum = pacc.tile([128, 257], FP)
            for k in range(NB):
                ksl = slice(k * 128, (k + 1) * 128)
                f = pp.tile([128, 128], FP)
                _mm(nc, f[:], xth0[:, ksl], cx0[:], start=True, stop=False)
                _mm(nc, f[:], xth1[:, ksl], cx1[:], start=False, stop=False)
                _mm(nc, f[:], onesh[:, ksl], cx2[:], start=False, stop=True)
                minv = sm.tile([128, 1], FP)
                nc.vector.tensor_reduce(minv[:], f[:], AX.X, OP.min)
                selt = sm.tile([128, 128], FP)
                nc.vector.tensor_scalar(selt[:], f[:], minv[:], None, OP.is_le)
                t3 = sm.tile([128, 128], FP)
                nc.vector.tensor_scalar(t3[:], selt[:], -BIGF, BIGF, OP.mult, OP.add)
                t4 = sm.tile([128, 128], FP)
                nc.vector.tensor_tensor(t4[:], t3[:], iot_c[:], OP.add)
                idxv = sm.tile([128, 1], FP)
                nc.vector.tensor_reduce(idxv[:], t4[:], AX.X, OP.min)
                oh = blk.tile([128, 128], FP)
                nc.vector.tensor_scalar(oh[:], iot_c[:], idxv[:], None, OP.is_equal)
                xrb = blk.tile([128, 256], FP)
                nc.gpsimd.dma_start(xrb[:], xr_d[ksl, :])
                xw = blk.tile([128, 257], FP)
                nc.scalar.activation(xw[:, 0:256], xrb[:], AF.Copy,
                                     scale=wcol[:, k : k + 1])
                nc.scalar.copy(xw[:, 256:257], wcol[:, k : k + 1])
                _mm(nc, msum[:], oh[:], xw[:], start=(k == 0), stop=(k == NB - 1),
                    skip_group_check=True)
            nc.scalar.copy(msumsb[:], msum[:])
            nc.gpsimd.dma_start(msum_d[:, :], msumsb[:])
    nc.compile()
    return nc


_cache = {}


def _graphs():
    if "a" not in _cache:
        _cache["a"] = build_a()
        _cache["b"] = build_b()
        _cache["c"] = build_c()
    return _cache["a"], _cache["b"], _cache["c"]


def _col(v):
    """[HALF] -> [128, NB] partition-major (col k holds rows k*128..k*128+127)."""
    return np.ascontiguousarray(v.reshape(NB, 128).T)


def _uncol(a):
    """[128, NB] -> [HALF]."""
    return np.ascontiguousarray(a.T).reshape(HALF)


def kernel(x, score_w, score_b, T=None, H=None, W=None, _timing=None):
    x = np.ascontiguousarray(np.asarray(x, np.float32))
    score_w = np.asarray(score_w, np.float32)
    score_b = np.asarray(score_b, np.float32)
    nca, ncb, ncc = _graphs()
    cores = list(range(8))

    sq = np.einsum("bnc,bnc->bn", x, x).astype(np.float32)
    w = np.exp(x @ score_w + score_b)[..., 0].astype(np.float32)
    noise = _noise((B, N))

    xt = [np.ascontiguousarray(x[b].T) for b in range(B)]
    rows = lambda c: slice((c % 2) * HALF, (c % 2 + 1) * HALF)

    in_a = []
    for c in cores:
        b = c // 2
        h = c % 2
        r = rows(c)
        # rotate the column space so this core's own rows sit at cols [0, HALF)
        sh = -h * HALF
        in_a.append({"xt": np.ascontiguousarray(np.roll(xt[b], sh, axis=1)),
                     "sqj": np.ascontiguousarray(np.roll(sq[b], sh))[None, :],
                     "lhs": np.ascontiguousarray(2.0 * xt[b][:, r]),
                     "sqi": _col(sq[b][r])})
    ra = run_bass_kernel_spmd(nca, in_a, core_ids=cores, **(_timing or {}))
    if _timing is not None:
        _timing.setdefault("_res", []).append(ra)

    sum9 = np.zeros((B, N), np.float32)
    emax = np.zeros((B, N), np.float32)
    for c in cores:
        b = c // 2
        r = rows(c)
        sum9[b, r] = _uncol(ra.results[c]["sum9"])
        emax[b, r] = -_uncol(ra.results[c]["eminneg"])
    density = (np.exp(-sum9 / (9.0 * 256.0)) + noise * 1e-6).astype(np.float32)
    d2max = (sq + emax).max(axis=1)
    dist_max = (np.sqrt(np.maximum(d2max, 0.0)) / 16.0).astype(np.float32)

    in_b = []
    for c in cores:
        b = c // 2
        h = c % 2
        r = rows(c)
        in_b.append({"ebuf": ra.results[c]["ebuf"],
                     "dens": np.ascontiguousarray(np.roll(density[b], -h * HALF))[None, :],
                     "densi": _col(density[b][r])})
    rb = run_bass_kernel_spmd(ncb, in_b, core_ids=cores, **(_timing or {}))
    if _timing is not None:
        _timing["_res"].append(rb)

    parent_e = np.zeros((B, N), np.float32)
    for c in cores:
        b = c // 2
        parent_e[b, rows(c)] = -_uncol(rb.results[c]["pmax"])
    cand = np.sqrt(np.maximum(sq + parent_e, 0.0)) / 16.0
    parent_dist = np.minimum(cand, dist_max[:, None])
    score = (parent_dist * density).astype(np.float32)

    in_c = []
    for c in cores:
        b = c // 2
        r = rows(c)
        idx = np.argsort(-score[b], kind="stable")[:M]
        cxa = np.concatenate([-2.0 * xt[b][:, idx], sq[b][idx][None, :]], 0)
        in_c.append({"xth": np.ascontiguousarray(xt[b][:, r]),
                     "cxa": np.ascontiguousarray(cxa),
                     "xr": np.ascontiguousarray(x[b, r]),
                     "wcol": _col(w[b, r])})
    rc = run_bass_kernel_spmd(ncc, in_c, core_ids=cores, **(_timing or {}))
    if _timing is not None:
        _timing["_res"].append(rc)

    out = np.zeros((B, M, C), np.float32)
    for b in range(B):
        tot = rc.results[2 * b]["msum"] + rc.results[2 * b + 1]["msum"]
        out[b] = tot[:, :C] / (tot[:, C:] + 1e-6)
    global _dbg
    _dbg = dict(sum9=sum9, emax=emax, density=density, dist_max=dist_max,
                parent_e=parent_e, score=score)
    return out
